# revision 1
# baseline (speedup 1.0000x reference)
"""Trainium2 Bass kernel for a 2-layer TransformerConv GNN + attention pooling.

Strategy: 64 equal graphs of 128 nodes; edges are within-graph. Shard 8
graphs per NeuronCore (batch sharding). Per graph, the scatter-softmax
attention over edges is computed DENSELY as masked attention with an edge
multiplicity matrix A[dst, src] (edge counts), so everything is PE matmuls:

  S = (Q/sqrt(C)) K^T  (per head)           -> PE
  P = A * exp(S - rowmax(S)); Z = rowsum(P) -> ACT/DVE
  agg = (P/Z) @ V                           -> PE (via P^T transpose)

Activations are kept TRANSPOSED [D, nodes] so natural-layout weights are
the stationary matmul operand. float32r matmuls run at 1 cycle/row for
free-dim >= 256 with near-fp32 precision; attention internals use bf16.
"""

import sys
import numpy as np

if "/opt/trn_rl_repo" not in sys.path:
    sys.path.insert(0, "/opt/trn_rl_repo")

import ml_dtypes  # noqa: F401
import concourse.bacc as bacc
import concourse.bass as bass
import concourse.mybir as mybir
import concourse.tile as tile
from concourse.bass_utils import run_bass_kernel_spmd

F32 = mybir.dt.float32
F32R = mybir.dt.float32r
BF16 = mybir.dt.bfloat16
AF = mybir.ActivationFunctionType
AX = mybir.AxisListType
ALU = mybir.AluOpType

# problem constants (hardcoded per contract)
B, L, D, H, E = 64, 128, 768, 2, 131072
N = B * L                 # 8192 nodes
C = D // H                # 384 per-head channels
N_CORES = 8
G = B // N_CORES          # 8 graphs per core
NPC = G * L               # 1024 nodes per core
KT = D // 128             # 6 k-tiles of 128
CT = C // 128             # 3 c-tiles per head
NCH = NPC // 512          # 2 node chunks of 512
SCALE = 1.0 / float(np.sqrt(C))

_CACHE = {}


def _bcast_cols(t, kt, col0, ngraph, rep):
    """AP over tile [128, KT, NPC] reading column col0 + g*L for each of
    `ngraph` graphs, each repeated `rep` times (step-0 inner dim)."""
    full = t[:]
    pstep = full.ap[0][0]
    off = full.offset + kt * NPC + col0
    return bass.AP(full.tensor, off, [[pstep, 128], [L, ngraph], [0, rep]])


def _build_program(repeat=1):
    nc = bacc.Bacc("TRN2", target_bir_lowering=False)

    # ---- DRAM I/O ----
    xT_d = nc.dram_tensor("xT", [D, NPC], F32R, kind="ExternalInput")
    a_d = nc.dram_tensor("acnt", [G, L, L], F32, kind="ExternalInput")
    wd = {}
    for l in ("1", "2"):
        for w in ("wq", "wk", "wv", "ws"):
            wd[w + l] = nc.dram_tensor(w + l, [D, D], F32R, kind="ExternalInput")
        for b in ("bq", "bk", "bs"):
            wd[b + l] = nc.dram_tensor(b + l, [D], F32, kind="ExternalInput")
        wd["bv" + l] = nc.dram_tensor("bv" + l, [D], F32R, kind="ExternalInput")
    atti_w = nc.dram_tensor("atti_w", [2 * D, D], F32R, kind="ExternalInput")
    atti_b = nc.dram_tensor("atti_b", [D], F32, kind="ExternalInput")
    atts_w = nc.dram_tensor("atts_w", [D, 1], F32, kind="ExternalInput")
    fc1_w = nc.dram_tensor("fc1_w", [D, D], F32R, kind="ExternalInput")
    fc1_b = nc.dram_tensor("fc1_b", [D], F32, kind="ExternalInput")
    fc2_w = nc.dram_tensor("fc2_w", [D, 3], F32R, kind="ExternalInput")
    fc2_b = nc.dram_tensor("fc2_b", [3], F32, kind="ExternalInput")
    eye_f = nc.dram_tensor("eye_f", [128, 128], F32R, kind="ExternalInput")
    ones_d = nc.dram_tensor("ones_d", [128, 128], F32R, kind="ExternalInput")
    eye_b = nc.dram_tensor("eye_b", [128, 128], BF16, kind="ExternalInput")
    tick_d = nc.dram_tensor("tick", [G, 3], F32, kind="ExternalInput")
    out_d = nc.dram_tensor("out", [G, 3], F32, kind="ExternalOutput")

    with tile.TileContext(nc) as tc, (
        tc.tile_pool(name="wpool", bufs=5)) as wp, (
        tc.tile_pool(name="act", bufs=4)) as actp, (
        tc.tile_pool(name="qk", bufs=1)) as qkp, (
        tc.tile_pool(name="small", bufs=1)) as sp, (
        tc.tile_pool(name="attn", bufs=4)) as ap_, (
        tc.tile_pool(name="psum", bufs=8, space=bass.MemorySpace.PSUM)) as pp:

        # ---- constants / small tiles (loaded once) ----
        tick_t = sp.tile([G, 3], F32, tag="tick")
        nc.scalar.dma_start(tick_t[:], tick_d[:, :])
        eyeF = sp.tile([128, 128], F32R, tag="eyeF")
        nc.scalar.dma_start(eyeF[:], eye_f[:, :])
        eyeB = sp.tile([128, 128], BF16, tag="eyeB")
        nc.scalar.dma_start(eyeB[:], eye_b[:, :])
        ones_row = sp.tile([1, 128], F32R, tag="ones_row")
        nc.scalar.dma_start(ones_row[:], ones_d[0:1, :])
        ones_row_f = sp.tile([1, 128], F32, tag="ones_row_f")
        nc.scalar.dma_start(ones_row_f[:], ones_d[0:1, :].bitcast(F32))
        ones_col_f = sp.tile([128, 1], F32, tag="ones_col_f")
        nc.scalar.dma_start(ones_col_f[:], ones_d[:, 0:1].bitcast(F32))

        def load_bias_cols(name, dram):
            t = sp.tile([128, KT], F32, tag="b_" + name)
            nc.scalar.dma_start(t[:], dram[:].rearrange("(t p) -> p t", p=128))
            return t

        bias_c = {}
        for l in ("1", "2"):
            for b in ("bq", "bk", "bs"):
                bias_c[b + l] = load_bias_cols(b + l, wd[b + l])
            t = sp.tile([1, D], F32R, tag="br_bv" + l)
            nc.scalar.dma_start(t[:], wd["bv" + l][:].rearrange("(a d) -> a d", a=1))
            bias_c["bv" + l] = t
            # pre-scaled bq for folding 1/sqrt(C) into q
            t = sp.tile([128, KT], F32, tag="bqs" + l)
            nc.vector.tensor_scalar_mul(t[:], bias_c["bq" + l][:], SCALE)
            bias_c["bqs" + l] = t
        attib_c = load_bias_cols("attib", atti_b)
        attsw_c = sp.tile([128, KT], F32, tag="attsw")
        nc.scalar.dma_start(attsw_c[:],
                          atts_w[:, :].rearrange("(t p) o -> p (t o)", p=128))
        attsw_b = sp.tile([128, KT], BF16, tag="attswb")
        nc.vector.tensor_copy(attsw_b[:], attsw_c[:])
        fc1b_c = load_bias_cols("fc1b", fc1_b)
        fc2w_c = sp.tile([128, KT, 3], F32R, tag="fc2w")
        nc.scalar.dma_start(fc2w_c[:],
                          fc2_w[:, :].rearrange("(t p) o -> p t o", p=128))
        fc2b_c = sp.tile([3, 1], F32, tag="fc2b")
        nc.scalar.dma_start(fc2b_c[:], fc2_b[:].rearrange("(o a) -> o a", a=1))

        def load_w(dram, rows=None):
            t = wp.tile([128, KT, D], F32R, tag="w")
            r0 = 0 if rows is None else rows[0]
            for kt in range(KT):
                nc.sync.dma_start(t[:, kt, :],
                                  dram[r0 + kt * 128:r0 + (kt + 1) * 128, :])
            return t

        def forward():
            # ---- interleave layer-1 wq with xT chunks: fast first-flight ----
            xT = [actp.tile([128, KT, 512], F32R, tag="act", name=f"xT{c}")
                  for c in range(NCH)]
            wq1 = wp.tile([128, KT, D], F32R, tag="w")
            for kt in range(KT):
                nc.sync.dma_start(wq1[:, kt, :],
                                  wd["wq1"][kt * 128:(kt + 1) * 128, :])
                for ch in range(NCH):
                    nc.sync.dma_start(
                        xT[ch][:, kt, :],
                        xT_d[kt * 128:(kt + 1) * 128, ch * 512:(ch + 1) * 512])
            # ln(edge count) [dst-local (partition), graph, src-local]
            A_sb = sp.tile([128, G, L], F32, tag="acnt")
            nc.scalar.dma_start(A_sb[:], a_d[:, :, :].rearrange("g p s -> p g s"))

            # =========== one TransformerConv layer ===========
            def conv_layer(lidx, actT, wq=None):
                l = str(lidx)
                if wq is None:
                    wq = load_w(wd["wq" + l])
                wk = load_w(wd["wk" + l])
                wv = load_w(wd["wv" + l])
                ws = load_w(wd["ws" + l])

                # --- qT, kT (transposed, bf16, q pre-scaled by 1/sqrt(C)) ---
                qT = qkp.tile([128, KT, NPC], BF16, tag="qT")
                kT = qkp.tile([128, KT, NPC], BF16, tag="kT")
                for w_sb, o_sb, scale, bias in (
                    (wq, qT, SCALE, bias_c["bqs" + l]),
                    (wk, kT, 1.0, bias_c["bk" + l]),
                ):
                    for dt in range(KT):
                        for ch in range(NCH):
                            ps = pp.tile([128, 512], F32, tag="bank")
                            for kt in range(KT):
                                nc.tensor.matmul(
                                    ps[:],
                                    w_sb[:, kt, dt * 128:(dt + 1) * 128],
                                    actT[ch][:, kt, :],
                                    start=(kt == 0), stop=(kt == KT - 1))
                            nc.vector.tensor_scalar(
                                o_sb[:, dt, ch * 512:(ch + 1) * 512], ps[:],
                                bias[:, dt:dt + 1], scale, ALU.add, ALU.mult)

                # --- V natural [node, D] bf16 (bias via ones-row matmul) ---
                v_sb = qkp.tile([128, G, D], BF16, tag="v")
                for g in range(G):
                    for chv in range(2):
                        ps = pp.tile([128, 384], F32, tag="bank")
                        for kt in range(KT):
                            nc.tensor.matmul(
                                ps[:],
                                actT[g // 4][:, kt,
                                             (g % 4) * 128:(g % 4 + 1) * 128],
                                wv[:, kt, chv * 384:(chv + 1) * 384],
                                start=(kt == 0), stop=False)
                        nc.tensor.matmul(
                            ps[:], ones_row[:],
                            bias_c["bv" + l][:, chv * 384:(chv + 1) * 384],
                            start=False, stop=True)
                        nc.vector.tensor_copy(
                            v_sb[:, g, chv * 384:(chv + 1) * 384], ps[:])

                # --- skip (x @ ws) into wide psum banks; attention adds in.
                # head h touches only dt in [h*CT, (h+1)*CT) -> park 3 banks
                # at a time and run that head's attention, keeping PSUM slack.
                hT = [actp.tile([128, KT, 512], F32R, tag="act",
                                name=f"hT{l}_{c}") for c in range(NCH)]
                for ch in range(NCH):
                    for h in range(H):
                        banks = []
                        for ct in range(CT):
                            dt = h * CT + ct
                            ps = pp.tile([128, 512], F32, tag="bank")
                            for kt in range(KT):
                                nc.tensor.matmul(
                                    ps[:],
                                    ws[:, kt, dt * 128:(dt + 1) * 128],
                                    actT[ch][:, kt, :],
                                    start=(kt == 0), stop=False)
                            banks.append(ps)

                        pns = []
                        for gl in range(4):
                            g = ch * 4 + gl
                            psS = pp.tile([128, 128], F32, tag="bank")
                            for ct in range(CT):
                                dti = h * CT + ct
                                nc.tensor.matmul(
                                    psS[:],
                                    qT[:, dti, g * 128:(g + 1) * 128],
                                    kT[:, dti, g * 128:(g + 1) * 128],
                                    start=(ct == 0), stop=False)
                            nc.tensor.matmul(psS[:], A_sb[:, g, :],
                                             eyeF[:].bitcast(F32),
                                             start=False, stop=True)
                            Pt = ap_.tile([128, 128], F32, tag="P")
                            Z = ap_.tile([128, 1], F32, tag="Z")
                            nc.scalar.activation(Pt[:], psS[:], AF.Exp,
                                                 accum_out=Z[:])
                            nc.vector.tensor_scalar_max(Z[:], Z[:], 1e-30)
                            r = ap_.tile([128, 1], F32, tag="r")
                            nc.vector.reciprocal(r[:], Z[:])
                            Pn = ap_.tile([128, 128], BF16, tag="Pn")
                            nc.vector.tensor_scalar(Pn[:], Pt[:], r[:], None,
                                                    ALU.mult)
                            pns.append(Pn)
                        for gl in range(4):
                            g = ch * 4 + gl
                            psT = pp.tile([128, 128], BF16, tag="bank")
                            nc.tensor.transpose(psT[:], pns[gl][:], eyeB[:])
                            PT = ap_.tile([128, 128], BF16, tag="PT")
                            nc.vector.tensor_copy(PT[:], psT[:])
                            for ct in range(CT):
                                dti = h * CT + ct
                                nc.tensor.matmul(
                                    banks[ct][:, gl * 128:(gl + 1) * 128],
                                    v_sb[:, g, dti * 128:(dti + 1) * 128],
                                    PT[:],
                                    start=False, stop=(gl == 3))

                        # --- evacuate: h = relu(skip + agg + bs) ---
                        for ct in range(CT):
                            dt = h * CT + ct
                            nc.scalar.activation(
                                hT[ch][:, dt, :], banks[ct][:],
                                AF.Relu, bias=bias_c["bs" + l][:, dt:dt + 1])
                return hT

            h1T = conv_layer(1, xT, wq=wq1)
            # prefetch pooling weights; DMA overlaps layer-2 compute
            attiT = load_w(atti_w, rows=(0, D))       # x_q part
            attiB = load_w(atti_w, rows=(D, 2 * D))   # h part
            h2T = conv_layer(2, h1T)

            # =========== attention pooling + head ===========

            # Qcols: first-node columns of h2T -> [128, KT, G]
            fc1w = load_w(fc1_w)  # prefetch
            Qcols = sp.tile([128, NCH, KT, 4], F32R, tag="Qcols")
            for ch in range(NCH):
                h2full = h2T[ch][:]
                qsrc = bass.AP(h2full.tensor, h2full.offset,
                               [[h2full.ap[0][0], 128], [512, KT], [L, 4]])
                nc.scalar.dma_start(Qcols[:, ch], qsrc)
            # xc = relu(h @ Wb + cTb[g]); cTb = x_q @ Wt + atti_b
            xcT = qkp.tile([128, KT, NPC], BF16, tag="v")  # reuse v slot
            cTb = sp.tile([128, KT, G], F32, tag="cTb")
            for ch in range(NCH):
                banks = []
                for dt in range(KT):
                    ps = pp.tile([128, 512], F32, tag="bank")
                    for kt in range(KT):
                        nc.tensor.matmul(
                            ps[:],
                            attiB[:, kt, dt * 128:(dt + 1) * 128],
                            h2T[ch][:, kt, :],
                            start=(kt == 0), stop=(kt == KT - 1))
                    banks.append(ps)
                if ch == 0:
                    for dt in range(KT):
                        psc = pp.tile([128, G], F32, tag="bank")
                        for kt in range(KT):
                            nc.tensor.matmul(
                                psc[:],
                                attiT[:, kt, dt * 128:(dt + 1) * 128],
                                Qcols[:, :, kt, :],
                                start=(kt == 0), stop=(kt == KT - 1))
                        nc.scalar.activation(cTb[:, dt, :], psc[:], AF.Identity,
                                             bias=attib_c[:, dt:dt + 1])
                for dt in range(KT):
                    for gl in range(4):
                        g = ch * 4 + gl
                        nc.vector.tensor_scalar(
                            xcT[:, dt, g * 128:(g + 1) * 128],
                            banks[dt][:, gl * 128:(gl + 1) * 128],
                            cTb[:, dt, g:g + 1], 0.0, ALU.add, ALU.max)

            # h2 natural (bf16) via PE transposes, for pooled = h2^T p
            h2n = actp.tile([128, G, D], BF16, tag="h2n", bufs=1)
            for g in range(G):
                for dt in range(KT):
                    ps = pp.tile([128, 128], F32R, tag="bank")
                    nc.tensor.transpose(
                        ps[:],
                        h2T[g // 4][:, dt, (g % 4) * 128:(g % 4 + 1) * 128],
                        eyeF[:])
                    nc.vector.tensor_copy(h2n[:, g, dt * 128:(dt + 1) * 128],
                                          ps[:].bitcast(F32))

            # batched per-graph score softmax -> pcols [128, G] bf16
            psSc = pp.tile([128, G], F32, tag="bank")
            for g in range(G):
                for kt in range(KT):
                    nc.tensor.matmul(psSc[:, g:g + 1],
                                     xcT[:, kt, g * 128:(g + 1) * 128],
                                     attsw_b[:, kt:kt + 1],
                                     start=(kt == 0), stop=(kt == KT - 1))
            Es = ap_.tile([128, G], F32, tag="Es")
            nc.scalar.activation(Es[:], psSc[:], AF.Exp)
            psZ = pp.tile([1, G], F32, tag="bank")
            nc.tensor.matmul(psZ[:], ones_col_f[:], Es[:], start=True, stop=True)
            Zs = ap_.tile([1, G], F32, tag="Zs")
            nc.scalar.copy(Zs[:], psZ[:])
            psZb = pp.tile([128, G], F32, tag="bank")
            nc.tensor.matmul(psZb[:], ones_row_f[:], Zs[:], start=True, stop=True)
            rp = ap_.tile([128, G], F32, tag="rp")
            nc.vector.reciprocal(rp[:], psZb[:])
            pcols = sp.tile([128, G], BF16, tag="pcols")
            nc.vector.tensor_mul(pcols[:], Es[:], rp[:])

            # pooledT[dout, g] = sum_n h2[n, dout] * p[n, g]
            pooledT = sp.tile([128, KT, G], F32R, tag="pooledT")
            for dt in range(KT):
                ps = pp.tile([128, G], F32, tag="bank")
                for g in range(G):
                    nc.tensor.matmul(ps[:, g:g + 1],
                                     h2n[:, g, dt * 128:(dt + 1) * 128],
                                     pcols[:, g:g + 1], start=True, stop=True)
                nc.scalar.copy(pooledT[:, dt, :], ps[:])

            # fc1 + tanh (transposed)
            z1 = sp.tile([128, KT, G], F32R, tag="z1")
            for dt in range(KT):
                ps = pp.tile([128, G], F32, tag="bank")
                for kt in range(KT):
                    nc.tensor.matmul(ps[:],
                                     fc1w[:, kt, dt * 128:(dt + 1) * 128],
                                     pooledT[:, kt, :],
                                     start=(kt == 0), stop=(kt == KT - 1))
                nc.scalar.activation(z1[:, dt, :], ps[:], AF.Tanh,
                                     bias=fc1b_c[:, dt:dt + 1])

            # fc2 -> [3, G] -> transpose -> log_softmax -> out
            psO = pp.tile([3, G], F32, tag="bank")
            for kt in range(KT):
                nc.tensor.matmul(psO[:], fc2w_c[:, kt, :], z1[:, kt, :],
                                 start=(kt == 0), stop=(kt == KT - 1))
            oT = sp.tile([3, G], F32, tag="oT")
            nc.scalar.activation(oT[:], psO[:], AF.Identity, bias=fc2b_c[:])
            psOt = pp.tile([G, 3], F32, tag="bank")
            nc.tensor.transpose(psOt[:], oT[:], eyeF[0:3, 0:3].bitcast(F32))
            nm = ap_.tile([G, 1], F32, tag="nm")
            nc.vector.reduce_max(nm[:], psOt[:], axis=AX.X, negate=True)
            eo = ap_.tile([G, 3], F32, tag="eo")
            zo = ap_.tile([G, 1], F32, tag="zo")
            nc.scalar.activation(eo[:], psOt[:], AF.Exp, bias=nm[:],
                                 accum_out=zo[:])
            lz = ap_.tile([G, 1], F32, tag="lz")
            nc.scalar.activation(lz[:], zo[:], AF.Ln)
            t1 = ap_.tile([G, 3], F32, tag="t1")
            nc.vector.tensor_scalar(t1[:], psOt[:], nm[:], None, ALU.add)
            ofin = ap_.tile([G, 3], F32, tag="ofin")
            nc.vector.tensor_scalar(ofin[:], t1[:], lz[:], None, ALU.subtract)
            nc.sync.dma_start(out_d[:, :], ofin[:])

        for _ in range(repeat):
            forward()

    nc.compile()
    return nc


def _get_program(repeat=1):
    key = ("nc", repeat)
    if key not in _CACHE:
        _CACHE[key] = _build_program(repeat)
    return _CACHE[key]


def make_in_maps(inputs):
    x = np.asarray(inputs["x"], np.float32)
    ei = np.asarray(inputs["edge_index"])
    src, dst = ei[0].astype(np.int64), ei[1].astype(np.int64)
    # A[graph, dst_local, src_local] edge counts; edges are within-graph
    flat = dst * L + (src % L)
    acnt = np.bincount(flat, minlength=N * L).reshape(B, L, L).astype(np.float32)
    with np.errstate(divide="ignore"):
        acnt = np.where(acnt > 0, np.log(acnt), np.float32(-1e30))
    acnt = np.ascontiguousarray(acnt.transpose(0, 2, 1)).astype(np.float32)

    shared = {}
    for l in ("1", "2"):
        for w in ("wq", "wk", "wv", "ws"):
            shared[w + l] = np.ascontiguousarray(np.asarray(inputs[w + l], np.float32))
        for b in ("bq", "bk", "bv", "bs"):
            shared[b + l] = np.ascontiguousarray(np.asarray(inputs[b + l], np.float32))
    for nme in ("atti_w", "atti_b", "atts_w", "fc1_w", "fc1_b", "fc2_w", "fc2_b"):
        shared[nme] = np.ascontiguousarray(np.asarray(inputs[nme], np.float32))
    shared["eye_f"] = np.eye(128, dtype=np.float32)
    shared["ones_d"] = np.ones((128, 128), np.float32)
    shared["eye_b"] = np.eye(128, dtype=ml_dtypes.bfloat16)

    in_maps = []
    for c in range(N_CORES):
        m = dict(shared)
        m["tick"] = np.zeros((G, 3), np.float32)
        m["xT"] = np.ascontiguousarray(x[c * NPC:(c + 1) * NPC].T)
        m["acnt"] = np.ascontiguousarray(acnt[c * G:(c + 1) * G])
        in_maps.append(m)
    return in_maps


def kernel(**inputs):
    nc = _get_program()
    in_maps = make_in_maps(inputs)
    res = run_bass_kernel_spmd(nc, in_maps, core_ids=list(range(N_CORES)))
    out = np.concatenate([res.results[c]["out"] for c in range(N_CORES)], axis=0)
    return out.astype(np.float32)



# revision 12
# speedup vs baseline: 2.5898x; 2.5898x over previous
"""Trainium2 Bass kernel: 2-layer TransformerConv GNN + attention pooling.

Strategy: batch-shard 8 graphs/core across 8 cores. Per graph the scatter
softmax is computed densely as masked attention with an edge-count matrix,
so everything is PE matmuls. The large projections run in fp8e4 with
DoubleRow perf mode (two 128-deep k-tiles contracted per instruction at
0.5 cycles/row -> 4x PE throughput vs f32r) and weights ship in fp8
(4x less HBM traffic). The attention P/V chain stays bf16 for DVE 2x
throughput and precision; Q/K are fp8 so the per-graph S = QK^T matmuls
also use DoubleRow.

Scale management: every fp8 tensor X is stored as X*s_X with a pow-2
per-tensor scale folded into the PSUM-evacuation op
(ACT: func(ps*scale + bias_col); DVE: (ps + b/s)*s). V is pre-scaled by
s_in*s_ws so the attention aggregate lands in the same scale as the skip
branch (they share a PSUM accumulation group). Folding the v-bias into
the output bias is exact because sum_s Pn[d,s] = 1.
"""

import sys
import numpy as np

if "/opt/trn_rl_repo" not in sys.path:
    sys.path.insert(0, "/opt/trn_rl_repo")

import ml_dtypes
import concourse.bacc as bacc
import concourse.bass as bass
import concourse.mybir as mybir
import concourse.tile as tile
from concourse.bass_utils import run_bass_kernel_spmd

F32 = mybir.dt.float32
F32R = mybir.dt.float32r
BF16 = mybir.dt.bfloat16
F8 = mybir.dt.float8e4
AF = mybir.ActivationFunctionType
AX = mybir.AxisListType
ALU = mybir.AluOpType
DR = mybir.MatmulPerfMode.DoubleRow
NP_F8 = ml_dtypes.float8_e4m3
NP_BF16 = ml_dtypes.bfloat16

# problem constants (hardcoded per contract)
B, L, D, H, E = 64, 128, 768, 2, 131072
N = B * L                 # 8192 nodes
C = D // H                # 384 per-head channels
N_CORES = 8
G = B // N_CORES          # 8 graphs per core
NPC = G * L               # 1024 nodes per core
KT = D // 128             # 6 k-tiles of 128
KP = KT // 2              # 3 k-tile pairs (DoubleRow)
CT = C // 128             # 3 c-tiles per head
NCH = NPC // 512          # 2 node chunks of 512
SQRTC = float(np.sqrt(C))

# static activation scales (fp8 headroom verified by offline emulation:
# max|q|/sqrtC*SQ ~ 10, max|k|*SK ~ 90, max|relu h|*SH ~ 98, max|xc|*SXC ~ 43)
SQ = 64.0
SK = 32.0
SH = 32.0
SXC = 64.0

_CACHE = {}

# packed f32 constant column layout
_OFF = {}
_c = 0
for _l in ("1", "2"):
    for _b in ("bqs", "bks", "bhs", "bqsd", "bksd"):
        _OFF[_b + _l] = _c
        _c += KT
_OFF["attib"] = _c; _c += KT
_OFF["fc1b"] = _c; _c += KT
_OFF["attsb"] = _c; _c += 1
_OFF["ones_col"] = _c; _c += 1
_OFF["fc2b"] = _c; _c += 1
_OFF["ones_row"] = _c; _c += 128
_CF = _c
_CB = 128 + 512 * 2 + 3 * KT   # eyeB | gmask0 | gmask1 | fc2w
_C8 = 128 + KT                 # eye8 | attsw


def _build_program(scales, repeat=1):
    (sx, swq1, swk1, swv1, sws1, swq2, swk2, swv2, sws2,
     swi, swatt, allbz) = scales
    nc = bacc.Bacc("TRN2", target_bir_lowering=False)

    sw = {"wq1": swq1, "wk1": swk1, "wv1": swv1, "ws1": sws1,
          "wq2": swq2, "wk2": swk2, "wv2": swv2, "ws2": sws2}
    s_in = {1: sx, 2: SH}

    # ---- DRAM I/O ----
    xT_d = nc.dram_tensor("xT", [D, NPC], F8, kind="ExternalInput")
    a_d = nc.dram_tensor("acnt", [G, L, L], BF16, kind="ExternalInput")
    wd = {}
    for l in ("1", "2"):
        for w in ("wq", "wk", "wv", "ws"):
            wd[w + l] = nc.dram_tensor(w + l, [D, D], F8, kind="ExternalInput")
    attiT_d = nc.dram_tensor("attiT", [D, D], F8, kind="ExternalInput")
    attiB_d = nc.dram_tensor("attiB", [D, D], F8, kind="ExternalInput")
    fc1w_d = nc.dram_tensor("fc1w", [D, D], BF16, kind="ExternalInput")
    constf_d = nc.dram_tensor("constf", [128, _CF], F32, kind="ExternalInput")
    constb_d = nc.dram_tensor("constb", [128, _CB], BF16,
                              kind="ExternalInput")
    const8_d = nc.dram_tensor("const8", [128, _C8], F8, kind="ExternalInput")
    tick_d = nc.dram_tensor("tick", [G, 3], F32, kind="ExternalInput")
    out_d = nc.dram_tensor("out", [G, 3], F32, kind="ExternalOutput")

    with tile.TileContext(nc) as tc, (
        tc.tile_pool(name="wpool", bufs=5)) as wp, (
        tc.tile_pool(name="act", bufs=1)) as actp, (
        tc.tile_pool(name="small", bufs=1)) as sp, (
        tc.tile_pool(name="attn", bufs=3)) as ap_, (
        tc.tile_pool(name="psum", bufs=8, space=bass.MemorySpace.PSUM)) as pp:

        # ---- constants: three packed DMAs on the idle Pool queue ----
        tick_t = sp.tile([G, 3], F32, tag="tick")
        nc.gpsimd.dma_start(tick_t[:], tick_d[:, :])
        constf = sp.tile([128, _CF], F32, tag="constf")
        nc.gpsimd.dma_start(constf[:], constf_d[:, :])
        constb = sp.tile([128, _CB], BF16, tag="constb")
        nc.gpsimd.dma_start(constb[:], constb_d[:, :])
        const8 = sp.tile([128, _C8], F8, tag="const8")
        nc.gpsimd.dma_start(const8[:], const8_d[:, :])

        def bcol(name, dt=0, n=1):
            o = _OFF[name] + dt
            return constf[:, o:o + n]

        eyeB = constb[:, 0:128]
        gmask = [constb[0:36, 128 + 512 * c:128 + 512 * (c + 1)]
                 for c in range(NCH)]
        fc2w_cols = 128 + 1024
        eye8 = const8[:, 0:128]
        ones_row_f = constf[0:1, _OFF["ones_row"]:_OFF["ones_row"] + 128]
        ones_col_f = bcol("ones_col")
        fc2b_c = constf[0:3, _OFF["fc2b"]:_OFF["fc2b"] + 1]

        def load_w(dram, tag="w"):
            t = wp.tile([128, KT, D], F8, tag=tag)
            nc.sync.dma_start(t[:], dram[:, :].rearrange("(t p) d -> p t d",
                                                         p=128))
            return t

        def forward():
            # ---- first flight: wq1 halves + xT chunks so PE starts early --
            wq1 = wp.tile([128, KT, D], F8, tag="w", name="wq1")
            for hf in range(2):
                nc.sync.dma_start(
                    wq1[:, :, hf * 384:(hf + 1) * 384],
                    wd["wq1"][:, hf * 384:(hf + 1) * 384].rearrange(
                        "(t p) d -> p t d", p=128))
            xT = [actp.tile([128, KT, 512], F8, tag=f"xT{c}", name=f"xT{c}")
                  for c in range(NCH)]
            for ch in range(NCH):
                nc.sync.dma_start(
                    xT[ch][:],
                    xT_d[:, ch * 512:(ch + 1) * 512].rearrange(
                        "(t p) n -> p t n", p=128))
            wk1 = load_w(wd["wk1"])
            # lnA * SQ*SK, layout [dst-local(P), graph, src-local]
            A_sb = sp.tile([128, G, L], BF16, tag="acnt")
            nc.gpsimd.dma_start(A_sb[:], a_d[:, :, :].rearrange("g p s -> p g s"))
            wv1 = load_w(wd["wv1"])
            ws1 = load_w(wd["ws1"])

            # =========== one TransformerConv layer ===========
            def conv_layer(lidx, actT, wq, wk, wv, ws):
                l = str(lidx)
                si = s_in[lidx]
                sq_scale = SQ / (si * sw["wq" + l] * SQRTC)
                sk_scale = SK / (si * sw["wk" + l])
                sv_scale = sw["ws" + l] / sw["wv" + l]
                sh_scale = SH / (si * sw["ws" + l])

                # --- qT, kT transposed fp8 (q pre-scaled by 1/sqrt(C)) ---
                qT = ap_.tile([128, KT, NPC], F8, tag="qT", bufs=1)
                kT = ap_.tile([128, KT, NPC], F8, tag="kT", bufs=1)
                for w_sb, o_sb, scl, bact, bdve in (
                    (wq, qT, sq_scale, "bqs" + l, "bqsd" + l),
                    (wk, kT, sk_scale, "bks" + l, "bksd" + l),
                ):
                    for dt in range(KT):
                        for ch in range(NCH):
                            ps = pp.tile([128, 512], F32, tag="bank")
                            for k in range(KP):
                                nc.tensor.matmul(
                                    ps[:],
                                    w_sb[:, 2 * k:2 * k + 2,
                                         dt * 128:(dt + 1) * 128],
                                    actT[ch][:, 2 * k:2 * k + 2, :],
                                    start=(k == 0), stop=(k == KP - 1),
                                    perf_mode=DR)
                            o_slice = o_sb[:, dt, ch * 512:(ch + 1) * 512]
                            if (dt + ch * KT) % 2 == 0:
                                nc.scalar.activation(
                                    o_slice, ps[:], AF.Identity,
                                    bias=bcol(bact, dt), scale=scl)
                            else:
                                nc.vector.tensor_scalar(
                                    o_slice, ps[:], bcol(bdve, dt),
                                    scl, ALU.add, ALU.mult)

                # --- V natural [node, G, D] bf16, scaled by s_in*s_ws ---
                v_sb = ap_.tile([128, G, D], BF16, tag="v", bufs=1)
                for g in range(G):
                    for chv in range(2):
                        ps = pp.tile([128, 384], F32, tag="bank")
                        for k in range(KP):
                            nc.tensor.matmul(
                                ps[:],
                                actT[g // 4][:, 2 * k:2 * k + 2,
                                             (g % 4) * 128:(g % 4 + 1) * 128],
                                wv[:, 2 * k:2 * k + 2,
                                   chv * 384:(chv + 1) * 384],
                                start=(k == 0), stop=(k == KP - 1),
                                perf_mode=DR)
                        o_slice = v_sb[:, g, chv * 384:(chv + 1) * 384]
                        if (g + chv) % 2 == 0:
                            nc.scalar.activation(o_slice, ps[:], AF.Identity,
                                                 scale=sv_scale)
                        else:
                            nc.vector.tensor_scalar_mul(o_slice, ps[:],
                                                        sv_scale)

                # --- skip into 3 parked banks; attention adds agg in ---
                # One-iteration software pipeline: while the exp/normalize
                # chain of iteration i-1 runs on ACT/DVE, the PE computes
                # skip+S of iteration i; transposes/agg of i-1 land after.
                hT = [actp.tile([128, KT, 512], F8, tag=f"hT{l}_{c}",
                                name=f"hT{l}_{c}") for c in range(NCH)]

                def make_skip(ch, h):
                    banks = []
                    for ct in range(CT):
                        dt = h * CT + ct
                        ps = pp.tile([128, 512], F32, tag="bank",
                                     name=f"skip{l}_{ch}_{h}_{ct}")
                        for k in range(KP):
                            nc.tensor.matmul(
                                ps[:],
                                ws[:, 2 * k:2 * k + 2,
                                   dt * 128:(dt + 1) * 128],
                                actT[ch][:, 2 * k:2 * k + 2, :],
                                start=(k == 0), stop=False,
                                perf_mode=DR)
                        banks.append(ps)
                    return banks

                def make_attn(ch, h):
                    dti = h * CT
                    psS = pp.tile([128, 512], F32, tag="bank",
                                  name=f"psS{l}_{ch}_{h}")
                    for gl in range(4):
                        g = ch * 4 + gl
                        gs = slice(g * 128, (g + 1) * 128)
                        out = psS[:, gl * 128:(gl + 1) * 128]
                        nc.tensor.matmul(
                            out, qT[:, dti:dti + 2, gs],
                            kT[:, dti:dti + 2, gs],
                            start=(gl == 0), stop=False, perf_mode=DR)
                        nc.tensor.matmul(
                            out, qT[:, dti + 2, gs], kT[:, dti + 2, gs],
                            start=False, stop=False)
                        nc.tensor.matmul(out, A_sb[:, g, :], eyeB,
                                         start=False, stop=(gl == 3))
                    Pt = ap_.tile([128, 512], BF16, tag="Pt")
                    nc.scalar.activation(Pt[:], psS[:], AF.Exp,
                                         scale=1.0 / (SQ * SK))
                    Zr = ap_.tile([128, 4, 1], F32, tag="Zr")
                    ptf = Pt[:]
                    pview = bass.AP(ptf.tensor, ptf.offset,
                                    [[ptf.ap[0][0], 128], [128, 4],
                                     [1, 128]])
                    nc.vector.tensor_reduce(Zr[:], pview, axis=AX.X,
                                            op=ALU.add)
                    rz = ap_.tile([128, 4, 1], F32, tag="rz")
                    nc.vector.reciprocal(rz[:], Zr[:])
                    Pn = ap_.tile([128, 512], BF16, tag="Pn")
                    for gl in range(4):
                        nc.vector.tensor_scalar_mul(
                            Pn[:, gl * 128:(gl + 1) * 128],
                            Pt[:, gl * 128:(gl + 1) * 128],
                            rz[:, gl, :])
                    return psS, Pn

                def flush_transp(st):
                    banks, psS, Pn, ch, h = st
                    tt = psS[:].bitcast(BF16)
                    for gl in range(4):
                        nc.tensor.matmul(
                            tt[:, gl * 128:(gl + 1) * 128],
                            Pn[:, gl * 128:(gl + 1) * 128], eyeB,
                            is_transpose=True,
                            start=(gl == 0), stop=(gl == 3))
                    PT4 = ap_.tile([128, 512], BF16, tag="PT")
                    nc.vector.tensor_copy(PT4[:], tt[:, 0:512])
                    return PT4

                def flush_agg(st, PT4):
                    banks, psS, Pn, ch, h = st
                    for gl in range(4):
                        g = ch * 4 + gl
                        for ct in range(CT):
                            dtv = h * CT + ct
                            nc.tensor.matmul(
                                banks[ct][:, gl * 128:(gl + 1) * 128],
                                v_sb[:, g, dtv * 128:(dtv + 1) * 128],
                                PT4[:, gl * 128:(gl + 1) * 128],
                                start=False, stop=(gl == 3))
                    for ct in range(CT):
                        dt = h * CT + ct
                        if allbz and (ch * 2 + h + ct) % 2 == 0:
                            nc.vector.tensor_scalar(
                                hT[ch][:, dt, :], banks[ct][:],
                                0.0, sh_scale, ALU.max, ALU.mult)
                        else:
                            nc.scalar.activation(
                                hT[ch][:, dt, :], banks[ct][:],
                                AF.Relu, bias=bcol("bhs" + l, dt),
                                scale=sh_scale)

                prev = None
                for ch in range(NCH):
                    for h in range(H):
                        banks = make_skip(ch, h)
                        if prev is not None:
                            pt4 = flush_transp(prev)
                        psS, Pn = make_attn(ch, h)
                        if prev is not None:
                            flush_agg(prev, pt4)
                        prev = (banks, psS, Pn, ch, h)
                pt4 = flush_transp(prev)
                flush_agg(prev, pt4)
                return hT

            h1T = conv_layer(1, xT, wq1, wk1, wv1, ws1)
            # prefetch layer-2 + pooling weights (DMA overlaps compute)
            wq2 = load_w(wd["wq2"])
            wk2 = load_w(wd["wk2"])
            wv2 = load_w(wd["wv2"])
            ws2 = load_w(wd["ws2"])
            h2T = conv_layer(2, h1T, wq2, wk2, wv2, ws2)

            # =========== attention pooling + head ===========
            attiB = load_w(attiB_d)   # h part
            attiT = load_w(attiT_d)   # x_q part
            fc1w = sp.tile([128, KT, D], BF16, tag="fc1w")
            nc.sync.dma_start(fc1w[:],
                              fc1w_d[:, :].rearrange("(t p) d -> p t d", p=128))

            sxc_scale = SXC / (SH * swi)

            # cTbT[g, d'] = (xq @ Wt)[g, d'] in psum scale (SH*swi), bf16.
            # The first-node columns enter as a strided gather stationary
            # operand; the per-graph bias is then broadcast along nodes into
            # the xc psum via a block-mask matmul (stationary cTbT).
            cTbT = sp.tile([36, D], BF16, tag="cTbT")
            nc.vector.memset(cTbT[:], 0)
            for half in range(2):
                for ch in range(NCH):
                    psCt = pp.tile([4, 384], F32, tag="bank",
                                   name=f"psCt{half}_{ch}")
                    h2f = h2T[ch][:]
                    for k in range(KP):
                        gat = bass.AP(h2f.tensor, h2f.offset + 2 * k * 512,
                                      [[h2f.ap[0][0], 128], [512, 2],
                                       [L, 4]])
                        nc.tensor.matmul(
                            psCt[:],
                            gat,
                            attiT[:, 2 * k:2 * k + 2,
                                  half * 384:(half + 1) * 384],
                            start=(k == 0), stop=(k == KP - 1),
                            perf_mode=DR)
                    nc.scalar.copy(
                        cTbT[ch * 32:ch * 32 + 4,
                             half * 384:(half + 1) * 384], psCt[:])

            # xcT = relu(h2 @ Wb + xq @ Wt + atti_b) * SXC (fp8, transposed)
            xcT = [actp.tile([128, KT, 512], F8, tag=f"xT{c}",
                             name=f"xcT{c}") for c in range(NCH)]
            for ch in range(NCH):
                for dt in range(KT):
                    ps = pp.tile([128, 512], F32, tag="bank")
                    for k in range(KP):
                        nc.tensor.matmul(
                            ps[:],
                            attiB[:, 2 * k:2 * k + 2, dt * 128:(dt + 1) * 128],
                            h2T[ch][:, 2 * k:2 * k + 2, :],
                            start=(k == 0), stop=False, perf_mode=DR)
                    nc.tensor.matmul(
                        ps[:], cTbT[:, dt * 128:(dt + 1) * 128],
                        gmask[ch], start=False, stop=True)
                    if allbz and dt % 2 == 0:
                        nc.vector.tensor_scalar(
                            xcT[ch][:, dt, :], ps[:],
                            0.0, sxc_scale, ALU.max, ALU.mult)
                    else:
                        nc.scalar.activation(
                            xcT[ch][:, dt, :], ps[:],
                            AF.Relu, bias=bcol("attib", dt),
                            scale=sxc_scale)

            # h2 natural bf16 via fp8 PE transposes (stride-2 psum out)
            h2n = ap_.tile([128, G, D], BF16, tag="v", bufs=1)  # reuse v slot
            for g in range(G):
                for half in range(2):
                    psF = pp.tile([128, 1024], F8, tag="bank")
                    pf = psF[:]
                    for i in range(CT):
                        dt = half * CT + i
                        outap = bass.AP(pf.tensor, pf.offset + i * 256,
                                        [[pf.ap[0][0], 128], [2, 128]])
                        nc.tensor.matmul(
                            outap,
                            h2T[g // 4][:, dt, (g % 4) * 128:(g % 4 + 1) * 128],
                            eye8, is_transpose=True,
                            start=(i == 0), stop=(i == CT - 1))
                    inview = bass.AP(pf.tensor, pf.offset,
                                     [[pf.ap[0][0], 128], [256, CT], [2, 128]])
                    if (g + half) % 2 == 0:
                        nc.scalar.copy(
                            h2n[:, g, half * 384:(half + 1) * 384], inview)
                    else:
                        nc.vector.tensor_copy(
                            h2n[:, g, half * 384:(half + 1) * 384], inview)

            # scores -> per-graph softmax -> pcols bf16
            psSc = pp.tile([128, G], F32, tag="bank")
            for g in range(G):
                ch, gl = g // 4, g % 4
                awf = const8[:]
                for k in range(KP):
                    mov = bass.AP(awf.tensor, awf.offset + 128 + 2 * k,
                                  [[awf.ap[0][0], 128], [1, 2], [1, 1]])
                    nc.tensor.matmul(
                        psSc[:, g:g + 1],
                        xcT[ch][:, 2 * k:2 * k + 2,
                                gl * 128:(gl + 1) * 128],
                        mov, start=(g == 0 and k == 0),
                        stop=(g == G - 1 and k == KP - 1), perf_mode=DR)
            Es = ap_.tile([128, G], F32, tag="Es")
            nc.scalar.activation(Es[:], psSc[:], AF.Exp, bias=bcol("attsb"),
                                 scale=1.0 / (SXC * swatt))
            psZ = pp.tile([1, G], F32, tag="bank")
            nc.tensor.matmul(psZ[:], ones_col_f, Es[:], start=True,
                             stop=True)
            Zs = ap_.tile([1, G], F32, tag="Zs")
            nc.scalar.copy(Zs[:], psZ[:])
            psZb = pp.tile([128, G], F32, tag="bank")
            nc.tensor.matmul(psZb[:], ones_row_f, Zs[:], start=True,
                             stop=True)
            rp = ap_.tile([128, G], F32, tag="rp")
            nc.vector.reciprocal(rp[:], psZb[:])
            pcols = sp.tile([128, G], BF16, tag="pcols")
            nc.vector.tensor_mul(pcols[:], Es[:], rp[:])

            # pooledT[dpart, dt, g] = sum_n h2n * pcols ; scale 1/SH
            psP = pp.tile([128, KT, G], F32, tag="bank")
            for dt in range(KT):
                for g in range(G):
                    nc.tensor.matmul(psP[:, dt, g:g + 1],
                                     h2n[:, g, dt * 128:(dt + 1) * 128],
                                     pcols[:, g:g + 1],
                                     start=(dt == 0 and g == 0),
                                     stop=(dt == KT - 1 and g == G - 1))
            pooledT = sp.tile([128, KT, G], BF16, tag="pooledT")
            nc.scalar.activation(pooledT[:], psP[:], AF.Identity,
                                 scale=1.0 / SH)

            # fc1 + tanh (bf16, transposed; single psum bank + one tanh)
            z1 = sp.tile([128, KT, G], BF16, tag="z1")
            psF1 = pp.tile([128, KT, G], F32, tag="bank")
            for dt in range(KT):
                for kt in range(KT):
                    nc.tensor.matmul(psF1[:, dt, :],
                                     fc1w[:, kt, dt * 128:(dt + 1) * 128],
                                     pooledT[:, kt, :],
                                     start=(dt == 0 and kt == 0),
                                     stop=(dt == KT - 1 and kt == KT - 1))
            if allbz:
                nc.scalar.activation(z1[:], psF1[:], AF.Tanh)
            else:
                for dt in range(KT):
                    nc.scalar.activation(z1[:, dt, :], psF1[:, dt, :],
                                         AF.Tanh, bias=bcol("fc1b", dt))

            # fc2 -> [3, G] -> transpose -> log_softmax -> out
            psO = pp.tile([3, G], F32, tag="bank")
            for kt in range(KT):
                nc.tensor.matmul(psO[:], constb[:, fc2w_cols + 3 * kt:fc2w_cols + 3 * kt + 3], z1[:, kt, :],
                                 start=(kt == 0), stop=(kt == KT - 1))
            oT = sp.tile([3, G], BF16, tag="oT")
            nc.scalar.activation(oT[:], psO[:], AF.Identity, bias=fc2b_c)
            psOt = pp.tile([G, 3], BF16, tag="bank")
            nc.tensor.transpose(psOt[:], oT[:], constb[0:3, 0:3])
            nm = ap_.tile([G, 1], F32, tag="nm")
            nc.vector.reduce_max(nm[:], psOt[:], axis=AX.X, negate=True)
            eo = ap_.tile([G, 3], F32, tag="eo")
            zo = ap_.tile([G, 1], F32, tag="zo")
            nc.scalar.activation(eo[:], psOt[:], AF.Exp, bias=nm[:],
                                 accum_out=zo[:])
            lz = ap_.tile([G, 1], F32, tag="lz")
            nc.scalar.activation(lz[:], zo[:], AF.Ln)
            t1 = ap_.tile([G, 3], F32, tag="t1")
            nc.vector.tensor_scalar(t1[:], psOt[:], nm[:], None, ALU.add)
            ofin = ap_.tile([G, 3], F32, tag="ofin")
            nc.vector.tensor_scalar(ofin[:], t1[:], lz[:], None, ALU.subtract)
            nc.sync.dma_start(out_d[:, :], ofin[:])

        for _ in range(repeat):
            forward()

    nc.compile()
    return nc


def _get_program(scales, repeat=1):
    key = (scales, repeat)
    if key not in _CACHE:
        _CACHE[key] = _build_program(scales, repeat)
    return _CACHE[key]


def _pow2_scale(a, target):
    m = float(np.abs(np.asarray(a, np.float32)).max())
    if m == 0:
        return 1.0
    return float(2.0 ** np.floor(np.log2(target / m)))


def _q8(a, scale):
    return np.asarray(np.asarray(a, np.float32) * scale, NP_F8)


def make_in_maps(inputs):
    x = np.asarray(inputs["x"], np.float32)
    ei = np.asarray(inputs["edge_index"])
    src, dst = ei[0].astype(np.int64), ei[1].astype(np.int64)
    # A[graph, dst_local, src_local] edge counts; edges are within-graph
    flat = dst * L + (src % L)
    acnt = np.bincount(flat, minlength=N * L).reshape(B, L, L).astype(np.float32)
    with np.errstate(divide="ignore"):
        acnt = np.where(acnt > 0, np.log(acnt), np.float32(-60.0))
    acnt = acnt * (SQ * SK)
    acnt = np.ascontiguousarray(acnt.transpose(0, 2, 1)).astype(NP_BF16)

    sx = _pow2_scale(x, 60.0)
    sws = {}
    for l in ("1", "2"):
        for w in ("wq", "wk", "wv", "ws"):
            sws[w + l] = _pow2_scale(inputs[w + l], 120.0)
    swi = _pow2_scale(inputs["atti_w"], 120.0)
    swatt = _pow2_scale(inputs["atts_w"], 120.0)
    allbz = all(
        float(np.abs(np.asarray(inputs[b])).max()) == 0.0
        for b in ("bq1", "bk1", "bv1", "bs1", "bq2", "bk2", "bv2", "bs2",
                  "atti_b", "atts_b"))
    scales = (sx, sws["wq1"], sws["wk1"], sws["wv1"], sws["ws1"],
              sws["wq2"], sws["wk2"], sws["wv2"], sws["ws2"], swi, swatt,
              allbz)

    def col(v):   # [D] -> [128, KT] column layout
        return np.ascontiguousarray(
            np.asarray(v, np.float32).reshape(KT, 128).T)

    shared = {}
    constf = np.zeros((128, _CF), np.float32)
    s_in = {"1": sx, "2": SH}
    for l in ("1", "2"):
        for w in ("wq", "wk", "wv", "ws"):
            shared[w + l] = _q8(inputs[w + l], sws[w + l])
        bq = np.asarray(inputs["bq" + l], np.float32)
        bk = np.asarray(inputs["bk" + l], np.float32)
        bv = np.asarray(inputs["bv" + l], np.float32)
        bs = np.asarray(inputs["bs" + l], np.float32)
        si = s_in[l]
        sq_scale = SQ / (si * sws["wq" + l] * SQRTC)
        sk_scale = SK / (si * sws["wk" + l])
        constf[:, _OFF["bqs" + l]:_OFF["bqs" + l] + KT] = col(bq * SQ / SQRTC)
        constf[:, _OFF["bks" + l]:_OFF["bks" + l] + KT] = col(bk * SK)
        constf[:, _OFF["bqsd" + l]:_OFF["bqsd" + l] + KT] = col(
            bq * SQ / SQRTC / sq_scale)
        constf[:, _OFF["bksd" + l]:_OFF["bksd" + l] + KT] = col(
            bk * SK / sk_scale)
        constf[:, _OFF["bhs" + l]:_OFF["bhs" + l] + KT] = col((bs + bv) * SH)
    constf[:, _OFF["attib"]:_OFF["attib"] + KT] = col(
        np.asarray(inputs["atti_b"]) * SXC)
    constf[:, _OFF["fc1b"]:_OFF["fc1b"] + KT] = col(inputs["fc1_b"])
    constf[:, _OFF["attsb"]] = float(np.asarray(inputs["atts_b"])[0])
    constf[:, _OFF["ones_col"]] = 1.0
    constf[0:3, _OFF["fc2b"]] = np.asarray(inputs["fc2_b"], np.float32)
    constf[:, _OFF["ones_row"]:_OFF["ones_row"] + 128] = 1.0
    shared["constf"] = constf

    constb = np.zeros((128, _CB), np.float32)
    constb[:, 0:128] = np.eye(128)
    for c in range(2):
        for gl in range(4):
            constb[c * 32 + gl, 128 + 512 * c + gl * 128:
                   128 + 512 * c + (gl + 1) * 128] = 1.0
    fc2w = np.asarray(inputs["fc2_w"], np.float32)   # [D, 3]
    constb[:, 128 + 1024:128 + 1024 + 3 * KT] = (
        fc2w.reshape(KT, 128, 3).transpose(1, 0, 2).reshape(128, 3 * KT))
    shared["constb"] = constb.astype(NP_BF16)

    const8 = np.zeros((128, _C8), np.float32)
    const8[:, 0:128] = np.eye(128)
    const8[:, 128:128 + KT] = col(
        np.asarray(inputs["atts_w"])[:, 0] * swatt)
    shared["const8"] = const8.astype(NP_F8)

    shared["attiT"] = _q8(np.asarray(inputs["atti_w"])[:D], swi)
    shared["attiB"] = _q8(np.asarray(inputs["atti_w"])[D:], swi)
    shared["fc1w"] = np.asarray(np.asarray(inputs["fc1_w"], np.float32),
                                NP_BF16)

    in_maps = []
    for c in range(N_CORES):
        m = dict(shared)
        m["tick"] = np.zeros((G, 3), np.float32)
        m["xT"] = np.ascontiguousarray(
            _q8(x[c * NPC:(c + 1) * NPC].T, sx))
        m["acnt"] = np.ascontiguousarray(acnt[c * G:(c + 1) * G])
        in_maps.append(m)
    return in_maps, scales


def kernel(**inputs):
    in_maps, scales = make_in_maps(inputs)
    nc = _get_program(scales)
    res = run_bass_kernel_spmd(nc, in_maps, core_ids=list(range(N_CORES)))
    out = np.concatenate([res.results[c]["out"] for c in range(N_CORES)],
                         axis=0)
    return out.astype(np.float32)
